# revision 9
# baseline (speedup 1.0000x reference)
"""Trainium2 Bass kernel for nn_LogicGatedSNN.

Computation (per reference):
  w       = (synapse_states > 50)            [O, I] binarize
  current = w @ spike_input                  [O]    GEMV
  v       = 0.8*membrane + current
  spikes  = (v >= vth)
  new_trace = clip(0.9*trace + outer(spikes, x), 0, 5)
  new_vth   = clip(vth + (spikes-0.05)*0.1, 0.5, 10)
  new_v     = v*(1-spikes)*0.2

Sharding: rows (out_features) split 8 ways across NeuronCores; x replicated.
All compute is local per core, no collectives.

Device mapping (per 128-row block):
  GEMV:  m[o,i] = (s[o,i] > xb[i]) where xb = 1000 - 950*x  (since s in [1,100),
         x in {0,1}: s > xb  <=>  (s > 50 and x == 1)).  One fused DVE
         tensor_tensor_reduce (is_gt + add-reduce) per I-chunk, chained via
         the reduce initial value -> current.
  trace: one custom fused DVE op  out = min(relu(t*0.9 + x*spikes_p), 5).
"""

import os
import sys
from contextlib import ExitStack

import numpy as np

for _p in ("/opt/trn_rl_repo", "/root/.axon_site/_ro/trn_rl_repo"):
    if os.path.isdir(_p) and _p not in sys.path:
        sys.path.append(_p)

import concourse.bacc as bacc
import concourse.bass as bass
import concourse.mybir as mybir
import concourse.tile as tile

N_CORES = 8
OUT_F = 8192
IN_F = 8192
O_SHARD = OUT_F // N_CORES          # 1024 rows per core
NB = O_SHARD // 128                 # 8 blocks of 128 partitions
F = 4096                            # free-dim chunk
NCHUNK = IN_F // F                  # 2 chunks

TRACE = False                       # set True (e.g. by test.py) for NTFF profiling
TRACE_DIR = "/root/problem/trace_out"
LAST = None                         # BassKernelResults of the last run

_SNN_OPS = None
_NC_CACHE = None


def _register_op(name, spec):
    from concourse import dve_ops
    from concourse.dve_spec import _has_src1 as has_src1, lower
    from concourse.dve_uop import DveOpSpec

    for op in dve_ops.OPS:
        if op.name == name:
            return op
    row = dve_ops._CUSTOM_DVE_ROW_BASE + len(dve_ops.OPS)
    shas = {}
    for ver in ("v3", "v4"):
        try:
            d = DveOpSpec(name=name, opcode=row, uops=lower(spec, ver=ver),
                          rd1_en=has_src1(spec))
            shas[ver] = d.sha(ver)
        except Exception:
            pass
    op = dve_ops.DveOp(name, spec, subdim=False, uops_sha=shas)
    dve_ops.OPS.append(op)
    dve_ops.CUSTOM_DVE_SPECS[name] = spec
    dve_ops._SUB_OPCODE_FOR_NAME[name] = row
    return op


def _get_snn_ops():
    """Register (once) the two fused DVE ops:
      SNN_TRACE_ANT: out = min(relu(in0*s0 + in1*s1), imm2)
      SNN_GEMV_ANT:  out = (in0 > in1); accum_out = s0 + sum(out)
    """
    global _SNN_OPS
    if _SNN_OPS is not None:
        return _SNN_OPS
    from operator import add

    from concourse.dve_spec import C0, C1, C2, Spec, Src0, Src1, minn, relu

    trace_spec = Spec(
        body=minn(relu(Src0 * C0 + Src1 * C1), C2),
        reference=lambda in0, in1, s0, s1, imm2: np.minimum(
            np.maximum(in0.astype(np.float32) * s0 + in1 * s1, 0.0), imm2
        ).astype(np.float32),
    )

    def _gemv_ref(in0, in1, s0, s1, imm2):
        b = (in0 > in1).astype(np.float32)
        return b, s0 + b.reshape(b.shape[0], -1).sum(axis=-1, keepdims=True)

    gemv_spec = Spec(
        body=(Src0 > Src1),
        accum=add,
        accum_init=C0,
        reference=_gemv_ref,
    )
    _SNN_OPS = (_register_op("SNN_TRACE_ANT", trace_spec),
                _register_op("SNN_GEMV_ANT", gemv_spec))
    return _SNN_OPS


def _build_nc():
    snn_op, gemv_op = _get_snn_ops()
    nc = bacc.Bacc("TRN2", target_bir_lowering=False, debug=False)
    f32 = mybir.dt.float32
    AO = mybir.AluOpType

    s_h = nc.dram_tensor("s_in", [O_SHARD, IN_F], f32, kind="ExternalInput")
    t_h = nc.dram_tensor("t_in", [O_SHARD, IN_F], f32, kind="ExternalInput")
    x_h = nc.dram_tensor("x_in", [IN_F], f32, kind="ExternalInput")
    mp_h = nc.dram_tensor("mp_in", [O_SHARD], f32, kind="ExternalInput")
    vth_h = nc.dram_tensor("vth_in", [O_SHARD], f32, kind="ExternalInput")
    spk_h = nc.dram_tensor("spk_out", [O_SHARD], f32, kind="ExternalOutput")
    v_h = nc.dram_tensor("v_out", [O_SHARD], f32, kind="ExternalOutput")
    tr_h = nc.dram_tensor("tr_out", [O_SHARD, IN_F], f32, kind="ExternalOutput")
    vt_h = nc.dram_tensor("vth_out", [O_SHARD], f32, kind="ExternalOutput")

    s_ap, t_ap, tr_ap = s_h.ap(), t_h.ap(), tr_h.ap()

    with ExitStack() as ctx:
        tc = ctx.enter_context(tile.TileContext(nc))
        cpool = ctx.enter_context(tc.tile_pool(name="const", bufs=1))
        spool = ctx.enter_context(tc.tile_pool(name="sload", bufs=2))
        tpool = ctx.enter_context(tc.tile_pool(name="tload", bufs=2))
        opool = ctx.enter_context(tc.tile_pool(name="ostore", bufs=2))
        scrpool = ctx.enter_context(tc.tile_pool(name="scr", bufs=1))
        smpool = ctx.enter_context(tc.tile_pool(name="small", bufs=4))

        # --- x broadcast to 128 partitions + compare threshold ------------
        # xb[i] = 1000 - 950*x[i]  ->  50 where x==1, 1000 where x==0.
        x_b = cpool.tile([128, IN_F], f32, tag="xrow")
        # partition-step-0 source AP: every partition reads the same x row
        nc.sync.dma_start(x_b[:, :], bass.AP(x_h, 0, [[0, 128], [1, IN_F]]))
        xb = cpool.tile([128, IN_F], f32, tag="xbrow")
        nc.vector.tensor_scalar(xb[:, :], x_b[:, :], -950.0, 1000.0,
                                AO.mult, AO.add)

        # --- per-block scalars in [128, NB] layout ------------------------
        mp_t = cpool.tile([128, NB], f32, tag="mp")
        nc.sync.dma_start(mp_t[:, :], mp_h.ap().rearrange("(b p) -> p b", p=128))
        vth_t = cpool.tile([128, NB], f32, tag="vth")
        nc.sync.dma_start(vth_t[:, :], vth_h.ap().rearrange("(b p) -> p b", p=128))
        spk_all = cpool.tile([128, NB], f32, tag="spk")
        nv_all = cpool.tile([128, NB], f32, tag="nv")
        nvt_all = cpool.tile([128, NB], f32, tag="nvt")

        for b in range(NB):
            rows = slice(b * 128, (b + 1) * 128)
            part = smpool.tile([128, NCHUNK], f32, tag="part")
            # --- Phase A: GEMV row-sums over binarized weights ------------
            for c in range(NCHUNK):
                cols = slice(c * F, (c + 1) * F)
                s_t = spool.tile([128, F], f32, tag="s")
                nc.sync.dma_start(s_t[:, :], s_ap[rows, cols])
                scr = scrpool.tile([128, F], f32, tag="scr")
                nc.vector._custom_dve(
                    gemv_op,
                    out=scr[:, :],
                    in0=s_t[:, :],
                    in1=xb[:, cols],
                    s0=0.0,
                    accum_out=part[:, c:c + 1],
                )
            curt = smpool.tile([128, 1], f32, tag="cur")
            nc.vector.tensor_tensor(curt[:, :], part[:, 0:1], part[:, 1:2], AO.add)
            cur = curt[:, :]
            # --- neuron state update ([128,1] ops) ------------------------
            v = smpool.tile([128, 1], f32, tag="v")
            nc.vector.tensor_scalar(v[:, :], mp_t[:, b:b + 1], 0.8, cur,
                                    AO.mult, AO.add)
            nc.vector.tensor_tensor(spk_all[:, b:b + 1], v[:, :],
                                    vth_t[:, b:b + 1], AO.is_ge)
            tmp = smpool.tile([128, 1], f32, tag="tmp")
            nc.vector.tensor_scalar(tmp[:, :], spk_all[:, b:b + 1], -0.2, 0.2,
                                    AO.mult, AO.add)
            nc.vector.tensor_tensor(nv_all[:, b:b + 1], v[:, :], tmp[:, :],
                                    AO.mult)
            tmp2 = smpool.tile([128, 1], f32, tag="tmp2")
            nc.vector.tensor_scalar(tmp2[:, :], spk_all[:, b:b + 1], 0.05, 0.1,
                                    AO.subtract, AO.mult)
            tmp3 = smpool.tile([128, 1], f32, tag="tmp3")
            nc.vector.tensor_tensor(tmp3[:, :], vth_t[:, b:b + 1], tmp2[:, :],
                                    AO.add)
            nc.vector.tensor_scalar(nvt_all[:, b:b + 1], tmp3[:, :], 0.5, 10.0,
                                    AO.max, AO.min)

            # --- Phase B: trace update ------------------------------------
            for c in range(NCHUNK):
                cols = slice(c * F, (c + 1) * F)
                t_t = tpool.tile([128, F], f32, tag="t")
                nc.sync.dma_start(t_t[:, :], t_ap[rows, cols])
                o_t = opool.tile([128, F], f32, tag="o")
                nc.vector._custom_dve(
                    snn_op, out=o_t[:, :], in0=t_t[:, :], in1=x_b[:, cols],
                    s0=0.9, s1=spk_all[:, b:b + 1], imm2=5.0)
                nc.scalar.dma_start(tr_ap[rows, cols], o_t[:, :])

        nc.sync.dma_start(spk_h.ap().rearrange("(b p) -> p b", p=128), spk_all[:, :])
        nc.sync.dma_start(v_h.ap().rearrange("(b p) -> p b", p=128), nv_all[:, :])
        nc.sync.dma_start(vt_h.ap().rearrange("(b p) -> p b", p=128), nvt_all[:, :])
    nc.compile()
    return nc


def kernel(spike_input, synapse_states, membrane_potential, adaptive_threshold,
           eligibility_trace):
    global _NC_CACHE, LAST
    x = np.ascontiguousarray(np.asarray(spike_input, dtype=np.float32))
    S = np.ascontiguousarray(np.asarray(synapse_states, dtype=np.float32))
    T = np.ascontiguousarray(np.asarray(eligibility_trace, dtype=np.float32))
    MP = np.ascontiguousarray(np.asarray(membrane_potential, dtype=np.float32))
    VT = np.ascontiguousarray(np.asarray(adaptive_threshold, dtype=np.float32))

    if _NC_CACHE is None:
        _NC_CACHE = _build_nc()
    nc = _NC_CACHE

    in_maps = []
    for k in range(N_CORES):
        r = slice(k * O_SHARD, (k + 1) * O_SHARD)
        in_maps.append({
            "s_in": np.ascontiguousarray(S[r]),
            "t_in": np.ascontiguousarray(T[r]),
            "x_in": x,
            "mp_in": np.ascontiguousarray(MP[r]),
            "vth_in": np.ascontiguousarray(VT[r]),
        })

    from concourse import bass_utils
    kwargs = {}
    if TRACE:
        bass_utils.upload_artifacts = lambda d: d  # no S3 in this container
        os.makedirs(TRACE_DIR, exist_ok=True)
        kwargs = dict(trace=True, tmpdir=TRACE_DIR)
    res = bass_utils.run_bass_kernel_spmd(nc, in_maps, core_ids=list(range(N_CORES)),
                                          **kwargs)
    LAST = res
    outs = res.results
    spikes = np.concatenate([outs[k]["spk_out"] for k in range(N_CORES)])
    new_v = np.concatenate([outs[k]["v_out"] for k in range(N_CORES)])
    new_tr = np.concatenate([outs[k]["tr_out"] for k in range(N_CORES)], axis=0)
    new_vt = np.concatenate([outs[k]["vth_out"] for k in range(N_CORES)])
    return (spikes, new_v, new_tr, new_vt)


# revision 17
# speedup vs baseline: 1.0551x; 1.0551x over previous
"""Trainium2 Bass kernel for nn_LogicGatedSNN.

Computation (per reference):
  w       = (synapse_states > 50)            [O, I] binarize
  current = w @ spike_input                  [O]    GEMV
  v       = 0.8*membrane + current
  spikes  = (v >= vth)
  new_trace = clip(0.9*trace + outer(spikes, x), 0, 5)
  new_vth   = clip(vth + (spikes-0.05)*0.1, 0.5, 10)
  new_v     = v*(1-spikes)*0.2

Sharding: rows (out_features) split 8 ways across NeuronCores; x replicated.
All compute is local per core, no collectives.

Device mapping (per 128-row block):
  GEMV:  m[o,i] = (s[o,i] > xb[i]) where xb = 1000 - 950*x  (since s in [1,100),
         x in {0,1}: s > xb  <=>  (s > 50 and x == 1)).  One fused DVE
         tensor_tensor_reduce (is_gt + add-reduce) per I-chunk, chained via
         the reduce initial value -> current.
  trace: one custom fused DVE op  out = min(relu(t*0.9 + x*spikes_p), 5).
"""

import os
import sys
from contextlib import ExitStack

import numpy as np

for _p in ("/opt/trn_rl_repo", "/root/.axon_site/_ro/trn_rl_repo"):
    if os.path.isdir(_p) and _p not in sys.path:
        sys.path.append(_p)

import concourse.bacc as bacc
import concourse.bass as bass
import concourse.mybir as mybir
import concourse.tile as tile

N_CORES = 8
OUT_F = 8192
IN_F = 8192
O_SHARD = OUT_F // N_CORES          # 1024 rows per core
NB = O_SHARD // 128                 # 8 blocks of 128 partitions
F = 2048                            # free-dim chunk
NCHUNK = IN_F // F                  # 4 chunks

TRACE = False                       # set True (e.g. by test.py) for NTFF profiling
TRACE_DIR = "/root/problem/trace_out"
LAST = None                         # BassKernelResults of the last run

_SNN_OPS = None
_NC_CACHE = None


def _register_op(name, spec):
    from concourse import dve_ops
    from concourse.dve_spec import _has_src1 as has_src1, lower
    from concourse.dve_uop import DveOpSpec

    for op in dve_ops.OPS:
        if op.name == name:
            return op
    row = dve_ops._CUSTOM_DVE_ROW_BASE + len(dve_ops.OPS)
    shas = {}
    for ver in ("v3", "v4"):
        try:
            d = DveOpSpec(name=name, opcode=row, uops=lower(spec, ver=ver),
                          rd1_en=has_src1(spec))
            shas[ver] = d.sha(ver)
        except Exception:
            pass
    op = dve_ops.DveOp(name, spec, subdim=False, uops_sha=shas)
    dve_ops.OPS.append(op)
    dve_ops.CUSTOM_DVE_SPECS[name] = spec
    dve_ops._SUB_OPCODE_FOR_NAME[name] = row
    return op


def _get_snn_ops():
    """Register (once) the two fused DVE ops:
      SNN_TRACE_ANT: out = min(relu(in0*s0 + in1*s1), imm2)
      SNN_GEMV_ANT:  out = (in0 > in1); accum_out = s0 + sum(out)
    """
    global _SNN_OPS
    if _SNN_OPS is not None:
        return _SNN_OPS
    from operator import add

    from concourse.dve_spec import C0, C1, C2, Spec, Src0, Src1, minn, relu

    trace_spec = Spec(
        body=minn(relu(Src0 * C0 + Src1 * C1), C2),
        reference=lambda in0, in1, s0, s1, imm2: np.minimum(
            np.maximum(in0.astype(np.float32) * s0 + in1 * s1, 0.0), imm2
        ).astype(np.float32),
    )

    def _gemv_ref(in0, in1, s0, s1, imm2):
        b = (in0 > in1).astype(np.float32)
        return b, s0 + b.reshape(b.shape[0], -1).sum(axis=-1, keepdims=True)

    gemv_spec = Spec(
        body=(Src0 > Src1),
        accum=add,
        accum_init=C0,
        reference=_gemv_ref,
    )
    _SNN_OPS = (_register_op("SNN_TRACE_ANT", trace_spec),
                _register_op("SNN_GEMV_ANT", gemv_spec))
    return _SNN_OPS


def _build_nc():
    snn_op, gemv_op = _get_snn_ops()
    nc = bacc.Bacc("TRN2", target_bir_lowering=False, debug=False)
    f32 = mybir.dt.float32
    AO = mybir.AluOpType

    s_h = nc.dram_tensor("s_in", [O_SHARD, IN_F], f32, kind="ExternalInput")
    t_h = nc.dram_tensor("t_in", [O_SHARD, IN_F], f32, kind="ExternalInput")
    x_h = nc.dram_tensor("x_in", [IN_F], f32, kind="ExternalInput")
    # vectors in [128, NB] partition-major layout; host transposes (free)
    mp_h = nc.dram_tensor("mp_in", [128, NB], f32, kind="ExternalInput")
    vth_h = nc.dram_tensor("vth_in", [128, NB], f32, kind="ExternalInput")
    spk_h = nc.dram_tensor("spk_out", [128, NB], f32, kind="ExternalOutput")
    v_h = nc.dram_tensor("v_out", [128, NB], f32, kind="ExternalOutput")
    tr_h = nc.dram_tensor("tr_out", [O_SHARD, IN_F], f32, kind="ExternalOutput")
    vt_h = nc.dram_tensor("vth_out", [128, NB], f32, kind="ExternalOutput")

    s_ap, t_ap, tr_ap = s_h.ap(), t_h.ap(), tr_h.ap()

    with ExitStack() as ctx:
        tc = ctx.enter_context(tile.TileContext(nc))
        cpool = ctx.enter_context(tc.tile_pool(name="const", bufs=1))
        spool = ctx.enter_context(tc.tile_pool(name="sload", bufs=3))
        tpool = ctx.enter_context(tc.tile_pool(name="tload", bufs=3))
        opool = ctx.enter_context(tc.tile_pool(name="ostore", bufs=3))
        scrpool = ctx.enter_context(tc.tile_pool(name="scr", bufs=2))
        smpool = ctx.enter_context(tc.tile_pool(name="small", bufs=4))

        # --- x broadcast to 128 partitions + compare threshold ------------
        # xb[i] = 1000 - 950*x[i]  ->  50 where x==1, 1000 where x==0.
        x_b = cpool.tile([128, IN_F], f32, tag="xrow")
        # partition-step-0 source AP: every partition reads the same x row
        nc.sync.dma_start(x_b[:, :], bass.AP(x_h, 0, [[0, 128], [1, IN_F]]))
        xb = cpool.tile([128, IN_F], f32, tag="xbrow")
        nc.vector.tensor_scalar(xb[:, :], x_b[:, :], -950.0, 1000.0,
                                AO.mult, AO.add)

        # --- per-block scalars in [128, NB] layout ------------------------
        mp_t = cpool.tile([128, NB], f32, tag="mp")
        nc.sync.dma_start(mp_t[:, :], mp_h.ap()[:, :])
        vth_t = cpool.tile([128, NB], f32, tag="vth")
        nc.sync.dma_start(vth_t[:, :], vth_h.ap()[:, :])
        spk_all = cpool.tile([128, NB], f32, tag="spk")
        nv_all = cpool.tile([128, NB], f32, tag="nv")
        nvt_all = cpool.tile([128, NB], f32, tag="nvt")

        for b in range(NB):
            rows = slice(b * 128, (b + 1) * 128)
            part = smpool.tile([128, NCHUNK], f32, tag="part")
            # --- Phase A: GEMV row-sums over binarized weights ------------
            for c in range(NCHUNK):
                cols = slice(c * F, (c + 1) * F)
                s_t = spool.tile([128, F], f32, tag="s")
                nc.sync.dma_start(s_t[:, :], s_ap[rows, cols])
                scr = scrpool.tile([128, F], f32, tag="scr")
                nc.vector._custom_dve(
                    gemv_op,
                    out=scr[:, :],
                    in0=s_t[:, :],
                    in1=xb[:, cols],
                    s0=0.0,
                    accum_out=part[:, c:c + 1],
                )
            curt = smpool.tile([128, 1], f32, tag="cur")
            ca = smpool.tile([128, 1], f32, tag="ca")
            cb = smpool.tile([128, 1], f32, tag="cb")
            nc.vector.tensor_tensor(ca[:, :], part[:, 0:1], part[:, 1:2], AO.add)
            nc.vector.tensor_tensor(cb[:, :], part[:, 2:3], part[:, 3:4], AO.add)
            nc.vector.tensor_tensor(curt[:, :], ca[:, :], cb[:, :], AO.add)
            cur = curt[:, :]
            # --- neuron state update ([128,1] ops) ------------------------
            v = smpool.tile([128, 1], f32, tag="v")
            nc.vector.tensor_scalar(v[:, :], mp_t[:, b:b + 1], 0.8, cur,
                                    AO.mult, AO.add)
            nc.vector.tensor_tensor(spk_all[:, b:b + 1], v[:, :],
                                    vth_t[:, b:b + 1], AO.is_ge)
            tmp = smpool.tile([128, 1], f32, tag="tmp")
            nc.vector.tensor_scalar(tmp[:, :], spk_all[:, b:b + 1], -0.2, 0.2,
                                    AO.mult, AO.add)
            nc.vector.tensor_tensor(nv_all[:, b:b + 1], v[:, :], tmp[:, :],
                                    AO.mult)
            tmp2 = smpool.tile([128, 1], f32, tag="tmp2")
            nc.vector.tensor_scalar(tmp2[:, :], spk_all[:, b:b + 1], 0.05, 0.1,
                                    AO.subtract, AO.mult)
            tmp3 = smpool.tile([128, 1], f32, tag="tmp3")
            nc.vector.tensor_tensor(tmp3[:, :], vth_t[:, b:b + 1], tmp2[:, :],
                                    AO.add)
            nc.vector.tensor_scalar(nvt_all[:, b:b + 1], tmp3[:, :], 0.5, 10.0,
                                    AO.max, AO.min)

            # --- Phase B: trace update ------------------------------------
            for c in range(NCHUNK):
                cols = slice(c * F, (c + 1) * F)
                t_t = tpool.tile([128, F], f32, tag="t")
                nc.sync.dma_start(t_t[:, :], t_ap[rows, cols])
                o_t = opool.tile([128, F], f32, tag="o")
                nc.vector._custom_dve(
                    snn_op, out=o_t[:, :], in0=t_t[:, :], in1=x_b[:, cols],
                    s0=0.9, s1=spk_all[:, b:b + 1], imm2=5.0)
                nc.scalar.dma_start(tr_ap[rows, cols], o_t[:, :])

        nc.sync.dma_start(spk_h.ap()[:, :], spk_all[:, :])
        nc.sync.dma_start(v_h.ap()[:, :], nv_all[:, :])
        nc.sync.dma_start(vt_h.ap()[:, :], nvt_all[:, :])
    nc.compile()
    return nc


def kernel(spike_input, synapse_states, membrane_potential, adaptive_threshold,
           eligibility_trace):
    global _NC_CACHE, LAST
    x = np.ascontiguousarray(np.asarray(spike_input, dtype=np.float32))
    S = np.ascontiguousarray(np.asarray(synapse_states, dtype=np.float32))
    T = np.ascontiguousarray(np.asarray(eligibility_trace, dtype=np.float32))
    MP = np.ascontiguousarray(np.asarray(membrane_potential, dtype=np.float32))
    VT = np.ascontiguousarray(np.asarray(adaptive_threshold, dtype=np.float32))

    if _NC_CACHE is None:
        _NC_CACHE = _build_nc()
    nc = _NC_CACHE

    in_maps = []
    for k in range(N_CORES):
        r = slice(k * O_SHARD, (k + 1) * O_SHARD)
        # [1024] -> [128, NB] with  arr2d[p, b] = vec[b*128 + p]
        in_maps.append({
            "s_in": np.ascontiguousarray(S[r]),
            "t_in": np.ascontiguousarray(T[r]),
            "x_in": x,
            "mp_in": np.ascontiguousarray(MP[r].reshape(NB, 128).T),
            "vth_in": np.ascontiguousarray(VT[r].reshape(NB, 128).T),
        })

    from concourse import bass_utils
    kwargs = {}
    if TRACE:
        bass_utils.upload_artifacts = lambda d: d  # no S3 in this container
        os.makedirs(TRACE_DIR, exist_ok=True)
        kwargs = dict(trace=True, tmpdir=TRACE_DIR)
    res = bass_utils.run_bass_kernel_spmd(nc, in_maps, core_ids=list(range(N_CORES)),
                                          **kwargs)
    LAST = res
    outs = res.results
    def _vec(name):
        # [128, NB] device layout -> [1024] with vec[b*128+p] = arr[p, b]
        return np.concatenate(
            [outs[k][name].T.reshape(-1) for k in range(N_CORES)])

    spikes = _vec("spk_out")
    new_v = _vec("v_out")
    new_tr = np.concatenate([outs[k]["tr_out"] for k in range(N_CORES)], axis=0)
    new_vt = _vec("vth_out")
    return (spikes, new_v, new_tr, new_vt)
